# revision 1
# baseline (speedup 1.0000x reference)
"""Trainium2 Bass kernel for nn_IntraAgg (GNN mean-neighbor aggregation).

reference:
    valid[b,k] = k < neigh_counts[b]
    out = relu( (sum_k valid[b,k] * features[neigh_idx[b,k]]) / neigh_counts[b] )

Strategy (8 NeuronCores, data-parallel over the batch):
  - shard neigh_idx/neigh_counts along B (8192 -> 1024 per core), replicate
    the features table.
  - host-side: cast indices to int32 and remap invalid (k >= count) slots to
    an out-of-bounds sentinel; precompute 1/count as f32.
  - per core, per 128-node block: one indirect (gather) DMA pulls all
    128*32 neighbor rows into a [128, 32*64] SBUF tile; out-of-bounds
    sentinel indices are skipped by the DMA bounds check, leaving zeros from
    an ACT-engine zero-fill.  A single strided DVE reduce sums over the 32
    neighbor slots, then one ACT op applies relu(x * (1/count)) and the
    result is DMAed out.
"""

import numpy as np

N_NODES = 1_000_000
FEAT_DIM = 64
BATCH = 8192
MAX_NEIGH = 32
N_CORES = 8
BLK = 128  # nodes per block (SBUF partition dim)

_KERNEL_CACHE = {}


def _split_multi_waits(nc):
    """walrus codegen accepts at most one sync-wait per instruction: hoist
    extra waits onto NoOp instructions inserted just before."""
    import bass_rust

    for fn in nc.m.functions:
        for bb in fn.blocks:
            new_list = []
            for inst in bb.instructions:
                si = inst.sync_info
                if si is not None and si.on_wait is not None and len(si.on_wait) > 1:
                    waits = list(si.on_wait)
                    for j, w in enumerate(waits[:-1]):
                        nop = bass_rust.InstNoOp(name=f"{inst.name}-sw{j}")
                        nop.engine = inst.engine
                        nop.sync_info = bass_rust.SyncInfo(on_wait=[w], on_update=[])
                        new_list.append(nop)
                    inst.sync_info = bass_rust.SyncInfo(
                        on_wait=[waits[-1]], on_update=list(si.on_update or [])
                    )
                new_list.append(inst)
            bb.instructions = new_list


def build_nc(n_nodes=N_NODES, b_loc=BATCH // N_CORES, k=MAX_NEIGH, d=FEAT_DIM,
             legalize=True, k_sched=None):
    """Build the per-core Bass program (SPMD: same program on every core)."""
    from concourse import bass, mybir
    from concourse.tile import TileContext

    assert b_loc % BLK == 0
    nblk = b_loc // BLK
    if k_sched is None:
        k_sched = [k] * nblk
    assert len(k_sched) == nblk and all(1 <= kj <= k for kj in k_sched)

    nc = bass.Bass()
    feat = nc.declare_dram_parameter("feat", [n_nodes, d], mybir.dt.float32,
                                     isOutput=False)
    idx = nc.declare_dram_parameter("idx", [b_loc, k], mybir.dt.int32,
                                    isOutput=False)
    recip = nc.declare_dram_parameter("recip", [b_loc, 1], mybir.dt.float32,
                                      isOutput=False)
    out = nc.declare_dram_parameter("out", [b_loc, d], mybir.dt.float32,
                                    isOutput=True)

    fp32 = mybir.dt.float32
    with TileContext(nc) as tc:
        with tc.tile_pool(name="const", bufs=1) as constp, \
             tc.tile_pool(name="idxp", bufs=5) as idxp, \
             tc.tile_pool(name="recp", bufs=5) as recp, \
             tc.tile_pool(name="gp", bufs=4) as gp, \
             tc.tile_pool(name="redp", bufs=3) as redp, \
             tc.tile_pool(name="outp", bufs=3) as outp:
            zcol = constp.tile([BLK, 1], fp32)
            nc.vector.memset(zcol[:], 0.0)
            bounds_reg = nc.gpsimd.to_reg(n_nodes - 1)
            for b in range(nblk):
                kj = k_sched[b]
                sl = slice(b * BLK, (b + 1) * BLK)
                it = idxp.tile([BLK, k], mybir.dt.int32, tag="it")
                nc.sync.dma_start(out=it[:, :kj], in_=idx[sl, :kj])
                rt = recp.tile([BLK, 1], fp32)
                nc.sync.dma_start(out=rt[:], in_=recip[sl, :])

                g = gp.tile([BLK, k * d], fp32, tag="g")
                # zero-fill on ACT (idle engine) so skipped gathers read as 0
                nc.scalar.activation(
                    out=g[:, :kj * d],
                    in_=zcol[:].to_broadcast([BLK, kj * d]),
                    func=mybir.ActivationFunctionType.Copy,
                )
                # HW consumes ONE index per partition per indirect DMA, so
                # gather neighbor k for all 128 nodes in one DMA; nodes are
                # count-sorted on the host so block b only needs k_sched[b]
                # DMAs.  Sentinel indices (> n_nodes-1) are skipped by the
                # bounds check, leaving the zero fill.
                for kk in range(kj):
                    nc.gpsimd.indirect_dma_start(
                        out=g[:, kk * d:(kk + 1) * d],
                        out_offset=None,
                        in_=feat[:, :],
                        in_offset=bass.IndirectOffsetOnAxis(
                            ap=it[:, kk:kk + 1], axis=0),
                        bounds_check=bounds_reg,
                        oob_is_err=False,
                    )
                red = redp.tile([BLK, d], fp32)
                nc.vector.tensor_reduce(
                    out=red[:],
                    in_=g[:, :kj * d].rearrange("p (k d) -> p d k", d=d),
                    axis=mybir.AxisListType.X,
                    op=mybir.AluOpType.add,
                )
                o = outp.tile([BLK, d], fp32)
                nc.scalar.activation(
                    out=o[:],
                    in_=red[:],
                    func=mybir.ActivationFunctionType.Relu,
                    scale=rt[:, :1],
                )
                nc.sync.dma_start(out=out[sl, :], in_=o[:])

    if legalize:
        _split_multi_waits(nc)
    return nc


def prep_core_inputs(features, neigh_idx, neigh_counts, n_cores=N_CORES):
    """Host-side sharding/remapping.  Nodes are sorted by descending neighbor
    count within each core so later blocks need fewer gather DMAs.

    Returns (in_maps, orders, k_sched): per-core input dicts, per-core node
    permutations (sorted -> original via out[order] = out_sorted), and the
    per-block gather-DMA counts (max over cores)."""
    n_nodes = features.shape[0]
    b = neigh_idx.shape[0]
    b_loc = b // n_cores
    k = neigh_idx.shape[1]
    nblk = b_loc // BLK

    idx32 = np.asarray(neigh_idx, dtype=np.int32).copy()
    counts = np.asarray(neigh_counts, dtype=np.int64)
    invalid = np.arange(k, dtype=np.int64)[None, :] >= counts[:, None]
    idx32[invalid] = n_nodes  # OOB sentinel -> skipped by bounds check
    recip = (1.0 / counts.astype(np.float64)).astype(np.float32)[:, None]

    feat = np.ascontiguousarray(np.asarray(features, dtype=np.float32))
    in_maps, orders = [], []
    k_sched = np.ones(nblk, dtype=np.int64)
    for c in range(n_cores):
        sl = slice(c * b_loc, (c + 1) * b_loc)
        cnt_c = counts[sl]
        order = np.argsort(-cnt_c, kind="stable")
        sorted_cnt = cnt_c[order]
        k_sched = np.maximum(
            k_sched, sorted_cnt.reshape(nblk, BLK).max(axis=1))
        orders.append(order)
        in_maps.append({
            "feat": feat,
            "idx": np.ascontiguousarray(idx32[sl][order]),
            "recip": np.ascontiguousarray(recip[sl][order]),
        })
    return in_maps, orders, tuple(int(x) for x in k_sched)


def kernel(features, neigh_idx, neigh_counts):
    from concourse.bass_utils import run_bass_kernel_spmd

    in_maps, orders, k_sched = prep_core_inputs(
        features, neigh_idx, neigh_counts)
    key = ("nc", N_NODES, BATCH // N_CORES, MAX_NEIGH, FEAT_DIM, k_sched)
    if key not in _KERNEL_CACHE:
        _KERNEL_CACHE[key] = build_nc(k_sched=list(k_sched))
    nc = _KERNEL_CACHE[key]

    res = run_bass_kernel_spmd(nc, in_maps, list(range(N_CORES)))
    b_loc = BATCH // N_CORES
    out = np.empty((BATCH, FEAT_DIM), dtype=np.float32)
    for c in range(N_CORES):
        out_c = np.empty((b_loc, FEAT_DIM), dtype=np.float32)
        out_c[orders[c]] = res.results[c]["out"]
        out[c * b_loc:(c + 1) * b_loc] = out_c
    return out

